# revision 1
# baseline (speedup 1.0000x reference)
"""MoE gate (DeepSeek-style noaux-free top-8 router) on 8 Trainium2 cores.

Full-input contract: kernel(x, gate_w) -> (topk_w, topk_idx, aux_loss).

Strategy (data-parallel over tokens, per the sharding hint):
  - Host: transpose each 1024-token shard of x to [7168, 1024] so the
    contraction dim lands on SBUF partitions; fold the 2.5 route scale
    into a replicated wT = (2.5*gate_w).T [7168, 256].
  - Device (per core): logits[t,e] via true-fp32 matmuls (4-pass PE mode --
    needed so top-8 indices match an fp32 reference), 56 K-chunks
    accumulated in PSUM per 128-token block.  Softmax numerator via ACT Exp
    (bias = -rowmax, accum_out = denominator), top-8 of the *logits* via the
    DVE max/max_index instructions (descending values, lowest-index-first on
    ties -- identical to jax.lax.top_k), renormalized top-8 weights, and a
    running [128,256] score accumulator that a final ones-matmul reduces
    across partitions into the per-expert score-sum partial.
  - Host: concat w/idx shards; p = sum(partials)/8192, f from a bincount of
    the indices, aux = 256 * sum(f*p).
"""

import os
import sys

import numpy as np

for _p in ("/opt/trn_rl_repo", "/root/.axon_site/_ro/trn_rl_repo"):
    if os.path.isdir(_p) and _p not in sys.path:
        sys.path.append(_p)

import concourse.bass as bass  # noqa: E402
import concourse.tile as tile  # noqa: E402
from concourse import bacc, mybir  # noqa: E402
from concourse.bass_utils import run_bass_kernel_spmd  # noqa: E402

DIM = 7168
N_EXPERTS = 256
TOP_K = 8
ROUTE_SCALE = 2.5
N_TOKENS = 8192
N_CORES = 8
TPC = N_TOKENS // N_CORES          # tokens per core = 1024
KC = DIM // 128                    # contraction chunks = 56
G = 256                            # tokens per x-DMA group
NG = TPC // G                      # groups per core = 4
NB = G // 128                      # 128-token blocks per group = 2
NBLK = TPC // 128                  # blocks per core = 8

F32 = mybir.dt.float32
U32 = mybir.dt.uint32


def build_nc():
    """Build + compile the per-core Bass program (SPMD: same program on all
    8 cores, different input data)."""
    nc = bacc.Bacc("TRN2", target_bir_lowering=False, debug=False,
                   num_devices=N_CORES)

    xt = nc.dram_tensor("xt", [DIM, TPC], F32, kind="ExternalInput")
    wt = nc.dram_tensor("wt", [DIM, N_EXPERTS], F32, kind="ExternalInput")
    w8_d = nc.dram_tensor("w8", [TPC, TOP_K], F32, kind="ExternalOutput")
    i8_d = nc.dram_tensor("i8", [TPC, TOP_K], U32, kind="ExternalOutput")
    pp_d = nc.dram_tensor("pp", [1, N_EXPERTS], F32, kind="ExternalOutput")

    xt_r = xt.rearrange("(k p) t -> p k t", p=128)   # [128, 56, 1024]
    wt_r = wt.rearrange("(k p) e -> p k e", p=128)   # [128, 56, 256]

    with tile.TileContext(nc) as tc:
        with (
            tc.tile_pool(name="wpool", bufs=1) as wpool,
            tc.tile_pool(name="xpool", bufs=2) as xpool,
            tc.tile_pool(name="spool", bufs=3) as spool,
            tc.tile_pool(name="acc", bufs=1) as accp,
            tc.tile_pool(name="tiny", bufs=12) as tiny,
            tc.tile_pool(name="k8", bufs=4) as k8p,
            tc.tile_pool(name="psl", bufs=4, space=bass.MemorySpace.PSUM) as psl,
            tc.tile_pool(name="psp", bufs=1, space=bass.MemorySpace.PSUM) as psp,
        ):
            wt_sb = wpool.tile([128, KC, N_EXPERTS], F32)
            nc.sync.dma_start(wt_sb[:], wt_r[:])

            ones = accp.tile([128, 1], F32)
            nc.vector.memset(ones[:], 1.0)
            sc_acc = accp.tile([128, N_EXPERTS], F32)
            nc.vector.memset(sc_acc[:], 0.0)

            for g in range(NG):
                xg = xpool.tile([128, KC, G], F32)
                nc.sync.dma_start(xg[:], xt_r[:, :, g * G:(g + 1) * G])

                for b in range(NB):
                    blk = g * NB + b
                    pl = psl.tile([128, N_EXPERTS], F32)
                    for k in range(KC):
                        nc.tensor.matmul(
                            pl[:],
                            xg[:, k, b * 128:(b + 1) * 128],
                            wt_sb[:, k, :],
                            start=(k == 0),
                            stop=(k == KC - 1),
                        )

                    # -max(logits) per token (bias for Exp)
                    negm = tiny.tile([128, 1], F32)
                    nc.vector.reduce_max(negm[:], pl[:],
                                         axis=mybir.AxisListType.X,
                                         negate=True)
                    # logits to SBUF for the top-8 unit
                    sl = spool.tile([128, N_EXPERTS], F32, tag="sl")
                    nc.vector.tensor_copy(sl[:], pl[:])
                    # exp(l - max), plus softmax denominator via accum_out
                    exps = spool.tile([128, N_EXPERTS], F32, tag="exps")
                    den = tiny.tile([128, 1], F32)
                    nc.scalar.activation(exps[:], pl[:],
                                         mybir.ActivationFunctionType.Exp,
                                         bias=negm[:], scale=1.0,
                                         accum_out=den[:])

                    # top-8 selection on the logits (exactly jax.lax.top_k)
                    v8 = k8p.tile([128, TOP_K], F32, tag="v8")
                    nc.vector.max(v8[:], sl[:])
                    i8 = k8p.tile([128, TOP_K], U32, tag="i8")
                    nc.vector.max_index(i8[:], v8[:], sl[:])

                    # renormalized top-8 weights: exp(v8-m) / sum
                    e8 = k8p.tile([128, TOP_K], F32, tag="e8")
                    s8 = tiny.tile([128, 1], F32)
                    nc.scalar.activation(e8[:], v8[:],
                                         mybir.ActivationFunctionType.Exp,
                                         bias=negm[:], scale=1.0,
                                         accum_out=s8[:])
                    rs8 = tiny.tile([128, 1], F32)
                    nc.vector.reciprocal(rs8[:], s8[:])
                    w8 = k8p.tile([128, TOP_K], F32, tag="w8")
                    nc.vector.tensor_scalar_mul(w8[:], e8[:], rs8[:])

                    # scores = exps/den, accumulated over blocks for p
                    rden = tiny.tile([128, 1], F32)
                    nc.vector.reciprocal(rden[:], den[:])
                    sc = spool.tile([128, N_EXPERTS], F32, tag="sc")
                    nc.vector.tensor_scalar_mul(sc[:], exps[:], rden[:])
                    nc.vector.tensor_add(sc_acc[:], sc_acc[:], sc[:])

                    nc.sync.dma_start(w8_d[blk * 128:(blk + 1) * 128, :], w8[:])
                    nc.sync.dma_start(i8_d[blk * 128:(blk + 1) * 128, :], i8[:])

            # cross-partition (token) reduce of the score accumulator
            pp_ps = psp.tile([128, N_EXPERTS], F32)
            nc.tensor.matmul(pp_ps[:1, :], ones[:], sc_acc[:],
                             start=True, stop=True)
            pp_sb = accp.tile([1, N_EXPERTS], F32)
            nc.vector.tensor_copy(pp_sb[:], pp_ps[:1, :])
            nc.sync.dma_start(pp_d[:], pp_sb[:])

    nc.compile()
    return nc


_NC = None


def _get_nc():
    global _NC
    if _NC is None:
        _NC = build_nc()
    return _NC


def make_in_maps(x, gate_w):
    x = np.asarray(x, dtype=np.float32)
    gate_w = np.asarray(gate_w, dtype=np.float32)
    assert x.shape == (N_TOKENS, DIM), x.shape
    assert gate_w.shape == (N_EXPERTS, DIM), gate_w.shape
    wt = np.ascontiguousarray((gate_w * np.float32(ROUTE_SCALE)).T)
    in_maps = []
    for c in range(N_CORES):
        shard = x[c * TPC:(c + 1) * TPC, :]
        in_maps.append({"xt": np.ascontiguousarray(shard.T), "wt": wt})
    return in_maps


def combine_results(results):
    topk_w = np.concatenate([r["w8"] for r in results], axis=0)
    topk_idx = np.concatenate([r["i8"] for r in results], axis=0).astype(np.int32)
    p_sum = np.sum(np.stack([r["pp"][0] for r in results]), axis=0,
                   dtype=np.float32)
    p = p_sum / np.float32(N_TOKENS)
    f = (np.bincount(topk_idx.ravel(), minlength=N_EXPERTS)
         .astype(np.float32) / np.float32(N_TOKENS))
    aux_loss = np.float32(np.sum(f * p, dtype=np.float32) * np.float32(N_EXPERTS))
    return topk_w.astype(np.float32), topk_idx, aux_loss


def kernel(x, gate_w):
    nc = _get_nc()
    in_maps = make_in_maps(x, gate_w)
    res = run_bass_kernel_spmd(nc, in_maps, list(range(N_CORES)))
    return combine_results(res.results)
